# revision 57
# baseline (speedup 1.0000x reference)
"""GPT2 self-attention on 8 trn2 NeuronCores (tensor-parallel).

Sharding: core c handles batch b = c//4 and head-group g = c%4 (4 of 16
heads = 256 of 1024 dims).

Per core:
  1. Q/K projection in fp8e4 DoubleRow (4x PE throughput): host splits
     x and 64*w_qkv into (hi, lo) e4m3 pairs; q/k = sum of the three
     component products hi*hi + hi*lo + lo*hi, each a DoubleRow matmul
     contracting two 128-d tiles at once. qkt [512 qk-dims, 2048 tokens].
  2. V projection likewise: [2048 tokens, 256 v-dims] (x^T tile as
     DoubleRow lhsT), stored per key-tile as [128, head, 65] with a ones
     column (col 64). v carries the 64x weight scale.
  3. Causal attention per head-pair in bf16, keys on PSUM partitions:
       S^T = K-tile.T @ Q-chunk (both heads into one 2-bank PSUM tile)
       -> merged exp(S/(8*64^2)) on ACT -> diag mask mult on GPSIMD
       -> probs bf16
       AV flipped: out[q-block 128, 65] += probs-block.T @ [V | 1]
       (col 64 = softmax denominator, landing per-query-partition)
     Normalize via DVE reciprocal + per-block tensor_scalar multiply.
  4. Transpose O_norm per 128-query block via DMA-transpose -> O^T [dims, q],
     then DVE-split O^T into (hi, lo) e4m3 for rounds 0-2.
  5. Partial out-projection z^T_partial [1024, 2048]: fp8 DoubleRow
     (both head-pairs in the two slots) for rounds 0-2, bf16 for the
     tail-critical round-3 epilogue. PSUM -> bf16 -> DRAM per [128, 512]
     tile (the output).

Host reorders/slices/casts inputs, and unshards by summing the four
tensor-parallel z^T partials per batch (f32), dividing out the 64^2
weight pre-scale, and transposing into [B, S, D]. b_qkv/b_out are zeros
by the problem spec and are folded out. Attention matmuls run bf16 with
fp32 PSUM accumulation; projections run fp8 DoubleRow with fp32 PSUM
accumulation.
"""

import numpy as np
import ml_dtypes
from contextlib import ExitStack

B, S, D, H = 2, 2048, 1024, 16
HD = 64            # head dim
NCORES = 8
HPC = 4            # heads per core
GD = HPC * HD      # 256 dims per core group
QW = 512           # query-chunk width
WS = 64.0          # weight pre-scale folded into exp-scale / host unshard
NKP = D // 256     # 4 DoubleRow contraction k-pairs over d_model

_CACHE = {}


def _build_program():
    import concourse.tile as tile
    from concourse import bacc, mybir

    bf16 = mybir.dt.bfloat16
    f8 = mybir.dt.float8e4
    f32 = mybir.dt.float32
    DR = mybir.MatmulPerfMode.DoubleRow

    nc = bacc.Bacc("TRN2", target_bir_lowering=False, debug=False,
                   num_devices=NCORES)

    # x8/wqk8/wv8: DoubleRow layouts [kpair*128, slot, comp(hi/lo), n]
    x8 = nc.dram_tensor("x8", [NKP * 128, 2, 2, S], f8, kind="ExternalInput").ap()
    wqk8 = nc.dram_tensor("wqk8", [NKP * 128, 2, 2, 2 * GD], f8, kind="ExternalInput").ap()
    wv8 = nc.dram_tensor("wv8", [NKP * 128, 2, 2, GD], f8, kind="ExternalInput").ap()
    # wo8: [128, slot(GD half), comp, D]; wo: bf16 for the epilogue round
    wo8 = nc.dram_tensor("wo8", [128, 2, 2, D], f8, kind="ExternalInput").ap()
    wo = nc.dram_tensor("wo", [GD, D], bf16, kind="ExternalInput").ap()
    mi = nc.dram_tensor("mi", [128, 256], bf16, kind="ExternalInput").ap()
    ztp = nc.dram_tensor("ztp", [D, S], bf16, kind="ExternalOutput").ap()

    NKT = S // 128          # 16 key tiles
    NQC = S // QW           # 4 query chunks
    ESCALE = 0.125 / (WS * WS)  # exp scale: 1/sqrt(HD) / weight-scale^2

    with tile.TileContext(nc) as tc, ExitStack() as ctx:
        persist = ctx.enter_context(tc.tile_pool(name="persist", bufs=1))
        # PSUM budget (8 banks): pscore 2x2 + pot 1x2 + pmisc 2x1 = 8
        pscore = ctx.enter_context(tc.tile_pool(name="pscore", bufs=2, space="PSUM"))
        pot = ctx.enter_context(tc.tile_pool(name="pot", bufs=1, space="PSUM"))
        pmisc = ctx.enter_context(tc.tile_pool(name="pmisc", bufs=2, space="PSUM"))
        prpool = ctx.enter_context(tc.tile_pool(name="prpool", bufs=36))
        onpool = ctx.enter_context(tc.tile_pool(name="onpool", bufs=3))
        ottpool = ctx.enter_context(tc.tile_pool(name="ottpool", bufs=3))
        recpool = ctx.enter_context(tc.tile_pool(name="recpool", bufs=3))
        zsbpool = ctx.enter_context(tc.tile_pool(name="zsbpool", bufs=10))
        warmpool = ctx.enter_context(tc.tile_pool(name="warmpool", bufs=1))

        x8_sb = [persist.tile([128, 2, 2, S], f8, tag=f"x8{p}", name=f"x8{p}") for p in range(NKP)]
        wqk8_sb = [persist.tile([128, 2, 2, 2 * GD], f8, tag=f"wqk8{p}", name=f"wqk8{p}") for p in range(NKP)]
        wv8_sb = [persist.tile([128, 2, 2, GD], f8, tag=f"wv8{p}", name=f"wv8{p}") for p in range(NKP)]
        wo8_sb = persist.tile([128, 2, 2, D], f8, tag="wo8", name="wo8_sb")
        wo_sb = [persist.tile([128, D], bf16, tag=f"wo{j}", name=f"wo{j}") for j in range(2)]
        mi_sb = persist.tile([128, 256], bf16, tag="mi", name="mi_sb")
        qkt_sb = [persist.tile([128, S], bf16, tag=f"qkt{m}", name=f"qkt{m}") for m in range(4)]
        v_sb = [persist.tile([128, HPC, HD + 1], bf16, tag=f"v{t}", name=f"v{t}") for t in range(NKT)]
        # O^T fp8 hi/lo per round, both pairs in the DoubleRow slot dim
        oth_sb = [persist.tile([128, 2, QW], f8, tag=f"oth{q}", name=f"oth{q}") for q in range(NQC - 1)]
        otl_sb = [persist.tile([128, 2, QW], f8, tag=f"otl{q}", name=f"otl{q}") for q in range(NQC - 1)]
        mtril_sb = mi_sb[:, 0:128]
        ident_sb = mi_sb[:, 128:256]

        # ---- PE p-state warmup ----
        # The cost model ramps PE to full clock only after ~3us of
        # continuous execution. Run throwaway matmuls on a memset tile
        # while the first DMAs land so real work starts near full speed.
        warm = warmpool.tile([128, 512], bf16, tag="warm", name="warm_sb")
        nc.vector.memset(warm[:], 0.0)
        wps = pmisc.tile([128, 512], f32, tag="misc", name="warm_ps")
        for _ in range(5):
            nc.tensor.matmul(wps[:], warm[:, 0:128], warm[:], start=True,
                             stop=True)
        # keep the BIR verifier happy: PSUM must have a reader
        nc.vector.tensor_copy(warm[:, 0:1], wps[:, 0:1])

        # ---- input loads ----
        # First 512 tokens of x + wqk feed qkt_chunk(.,0) immediately.
        # Fan the 8 critical dispatches over all three DMA-capable queues
        # (each dispatch serializes ~790ns on its queue) so the last
        # (x8[p], wqk8[p]) pair lands ~4.5us instead of ~5.5us.
        def xload(p, sl):
            return dict(out=x8_sb[p][:, :, :, sl], in_=x8[p * 128:(p + 1) * 128, :, :, sl])

        def wqkload(p):
            return dict(out=wqk8_sb[p][:], in_=wqk8[p * 128:(p + 1) * 128])

        c0 = slice(0, QW)
        nc.sync.dma_start(**xload(0, c0))
        nc.gpsimd.dma_start(**wqkload(0))
        nc.scalar.dma_start(**xload(1, c0))
        nc.sync.dma_start(**wqkload(1))
        nc.gpsimd.dma_start(**xload(2, c0))
        nc.scalar.dma_start(**wqkload(2))
        nc.sync.dma_start(**xload(3, c0))
        nc.gpsimd.dma_start(**wqkload(3))
        nc.scalar.dma_start(out=mi_sb[:], in_=mi[:])
        for p in range(NKP):
            nc.sync.dma_start(**xload(p, slice(QW, S)))
            nc.gpsimd.dma_start(out=wv8_sb[p][:], in_=wv8[p * 128:(p + 1) * 128])
        nc.gpsimd.dma_start(out=wo8_sb[:], in_=wo8[:])
        for j in range(2):
            nc.gpsimd.dma_start(out=wo_sb[j][:], in_=wo[j * 128:(j + 1) * 128, :])

        # ---- projection helpers (PE fill work, fp8 DoubleRow) ----
        # terms: (w_hi, x_hi), (w_hi, x_lo), (w_lo, x_hi)
        TERMS = [(0, 0), (0, 1), (1, 0)]

        def qkt_chunk(m, n, lo=0, hi=QW, act_copy=False):
            ps = pmisc.tile([128, QW], f32, tag="misc", name="qkt_ps")[:, 0:hi - lo]
            k = 0
            for wc, xc in TERMS:
                for p in range(NKP):
                    nc.tensor.matmul(
                        ps,
                        wqk8_sb[p][:, :, wc, m * 128:(m + 1) * 128],
                        x8_sb[p][:, :, xc, n * QW + lo:n * QW + hi],
                        start=(k == 0), stop=(k == 3 * NKP - 1),
                        perf_mode=DR,
                    )
                    k += 1
            if act_copy:
                # preamble only: ACT is idle pre-exp, so the two first
                # chunk copies run in parallel on DVE + ACT
                nc.scalar.activation(qkt_sb[m][:, n * QW + lo:n * QW + hi],
                                     ps, mybir.ActivationFunctionType.Copy)
            else:
                nc.vector.tensor_copy(qkt_sb[m][:, n * QW + lo:n * QW + hi], ps)

        def v_tile(t):
            ps = pmisc.tile([128, GD], f32, tag="misc", name="v_ps")
            k = 0
            for wc, xc in TERMS:
                for p in range(NKP):
                    nc.tensor.matmul(
                        ps[:, 0:GD],
                        x8_sb[p][:, :, xc, t * 128:(t + 1) * 128],
                        wv8_sb[p][:, :, wc, :],
                        start=(k == 0), stop=(k == 3 * NKP - 1),
                        perf_mode=DR,
                    )
                    k += 1
            nc.vector.tensor_copy(
                v_sb[t][:, :, 0:HD],
                ps[:, 0:GD].rearrange("p (h d) -> p h d", h=HPC),
            )
            nc.vector.memset(v_sb[t][:, :, HD:HD + 1], 1.0)

        ott_of = {}
        zsplit = {}

        def zp_step(qc, ct, epilogue=False):
            """One out-proj column tile: z^T[ct*128:+128, qc*512:+512]."""
            if epilogue:
                if ct in zsplit:
                    # pair0 contribution was accumulated mid-round; only
                    # pair1 remains on the tail-critical path
                    ps = zsplit[ct]
                    nc.tensor.matmul(
                        ps,
                        wo_sb[1][:, ct * 128:(ct + 1) * 128],
                        ott_of[(1, qc)][:],
                        start=False, stop=True,
                    )
                else:
                    pool, tag = ((pmisc, "misc") if ct % 2 == 0
                                 else (pscore, "sc"))
                    ps = pool.tile([128, QW], f32, tag=tag, name="zp_ps")[:]
                    for pair in (0, 1):
                        nc.tensor.matmul(
                            ps,
                            wo_sb[pair][:, ct * 128:(ct + 1) * 128],
                            ott_of[(pair, qc)][:],
                            start=(pair == 0), stop=(pair == 1),
                        )
            else:
                ps = pmisc.tile([128, QW], f32, tag="misc", name="zp_ps")[:]
                # fp8 DoubleRow: both pairs in the slots, 3 hi/lo terms
                zterms = [(0, oth_sb[qc]), (0, otl_sb[qc]), (1, oth_sb[qc])]
                for k, (wc, ot8) in enumerate(zterms):
                    nc.tensor.matmul(
                        ps,
                        wo8_sb[:, :, wc, ct * 128:(ct + 1) * 128],
                        ot8[:],
                        start=(k == 0), stop=(k == 2),
                        perf_mode=DR,
                    )
            zsb = zsbpool.tile([128, QW], bf16, tag="zsb", name="zsb")
            if epilogue and ct % 2 == 0:
                nc.scalar.activation(zsb[:], ps,
                                     mybir.ActivationFunctionType.Copy)
            else:
                nc.vector.tensor_copy(zsb[:], ps)
            if epilogue:
                eng = (nc.sync, nc.gpsimd, nc.sync, nc.sync)[ct % 4]
            else:
                eng = nc.gpsimd if ct % 2 == 0 else nc.sync
            eng.dma_start(
                out=ztp[ct * 128:(ct + 1) * 128, qc * QW:(qc + 1) * QW],
                in_=zsb[:])

        # ---- attention ----
        def emit_scores(pair, qc, kt, prs):
            qstart = qc * QW
            j = kt - 4 * qc
            qoff = max(0, 128 * j)
            sp = pscore.tile([128, 1024], f32, tag="sc", name="sc_ps")
            pr = prpool.tile([128, 1024], bf16, tag="pr", name="pr_sb")
            for hh in range(2):
                base = 64 * hh
                nc.tensor.matmul(
                    sp[:, 512 * hh + qoff:512 * hh + 512],
                    qkt_sb[2 + pair][base:base + 64, kt * 128:(kt + 1) * 128],
                    qkt_sb[pair][base:base + 64, qstart + qoff:qstart + QW],
                    start=True, stop=True,
                )
            sp3 = sp[:].rearrange("p (h q) -> p h q", h=2)
            pr3 = pr[:].rearrange("p (h q) -> p h q", h=2)
            nc.scalar.activation(
                pr3[:, :, qoff:QW], sp3[:, :, qoff:QW],
                mybir.ActivationFunctionType.Exp,
                scale=ESCALE,
            )
            if j >= 0:
                # causal mask: zero future-key probs in the diagonal tile
                # (on GPSIMD: keeps DVE free for copies/norms)
                nc.gpsimd.tensor_tensor(
                    out=pr3[:, :, qoff:qoff + 128],
                    in0=pr3[:, :, qoff:qoff + 128],
                    in1=mtril_sb.rearrange("p (o c) -> p o c", o=1).to_broadcast([128, 2, 128]),
                    op=mybir.AluOpType.mult,
                )
            prs[kt] = pr

        def av_block(pair, qc, ot, prs, hh, qb):
            """One (head, query-block) AV accumulation group: consecutive
            matmuls over its key tiles (one open PSUM group per bank)."""
            blk = hh * 4 + qb
            last = 4 * qc + qb
            for kt in range(last + 1):
                pr3 = prs[kt][:].rearrange("p (h q) -> p h q", h=2)
                nc.tensor.matmul(
                    ot[:, 128 * blk:128 * blk + HD + 1],
                    pr3[:, hh, qb * 128:(qb + 1) * 128],
                    v_sb[kt][:, 2 * pair + hh, :],
                    start=(kt == 0), stop=(kt == last),
                )

        def norm_transpose(pair, qc, ot, pe_transpose=False):
            ot3 = ot[:].rearrange("p (b q) -> p b q", b=8)
            rec = recpool.tile([128, 8], f32, tag="rec", name="rec_sb")
            nc.vector.reciprocal(rec[:], ot3[:, :, HD:HD + 1])
            onorm = onpool.tile([128, QW], bf16, tag="on", name="on_sb")
            nc.vector.tensor_tensor(
                out=onorm[:].rearrange("p (qb hh d) -> p hh qb d", qb=4, hh=2),
                in0=ot[:].rearrange("p (hh qb c) -> p hh qb c", hh=2, qb=4)[:, :, :, 0:HD],
                in1=rec[:].rearrange("p (hh qb) -> p hh qb", hh=2).to_broadcast([128, 2, 4, HD]),
                op=mybir.AluOpType.mult,
            )
            ott = ottpool.tile([128, QW], bf16, tag=f"ott{pair}", name="ott_sb")
            if pe_transpose:
                # tail-critical: PE is idle here and skips the DMA-queue
                # latency
                tp = pmisc.tile([128, QW], bf16, tag="misc", name="tp_ps")
                for qb in range(4):
                    nc.tensor.transpose(
                        tp[:, qb * 128:(qb + 1) * 128],
                        onorm[:, qb * 128:(qb + 1) * 128],
                        ident_sb)
                nc.vector.tensor_copy(ott[:], tp[:])
            else:
                for qb in range(4):
                    nc.sync.dma_start_transpose(
                        ott[:, qb * 128:(qb + 1) * 128],
                        onorm[:, qb * 128:(qb + 1) * 128])
            ott_of[(pair, qc)] = ott
            if qc < NQC - 1:
                # fp8 hi/lo split of O^T for the DoubleRow out-projection
                nc.vector.tensor_copy(oth_sb[qc][:, pair, :], ott[:])
                nc.vector.tensor_tensor(
                    out=otl_sb[qc][:, pair, :],
                    in0=ott[:],
                    in1=oth_sb[qc][:, pair, :],
                    op=mybir.AluOpType.subtract,
                )

        # ---- main schedule ----
        qkt_chunk(0, 0)
        qkt_chunk(2, 0)

        prs0 = {}
        for qc in range(NQC):
            nkt = 4 * qc + 4
            fills = []
            if qc == 0:
                fills.append(lambda: qkt_chunk(1, 0))
                fills.append(lambda: qkt_chunk(3, 0))
                for t in range(4):
                    fills.append(lambda t=t: v_tile(t))
                fills.append(lambda: qkt_chunk(0, 1))
                fills.append(lambda: qkt_chunk(2, 1))
            else:
                # v tiles for THIS round's AV phase: consumed during pair0
                # scores
                for t in range(4 * qc, 4 * qc + 4):
                    fills.append(lambda t=t: v_tile(t))
                if qc < NQC - 1:
                    fills.append(lambda n=qc + 1: qkt_chunk(0, n))
                    fills.append(lambda n=qc + 1: qkt_chunk(2, n))
            if qc == NQC - 1:
                fills.append(lambda n=qc: qkt_chunk(1, n))
                fills.append(lambda n=qc: qkt_chunk(3, n))
            elif qc < NQC - 2:
                fills.append(lambda n=qc + 1: qkt_chunk(1, n))
                fills.append(lambda n=qc + 1: qkt_chunk(3, n))
            # all zp waves deferred to round 3 — the only PE-starved round
            if qc == NQC - 1:
                for q in range(NQC - 1):
                    for ct in range(8):
                        fills.append(lambda c=ct, q=q: zp_step(q, c))

            prs1 = {}
            # pair0 scores (ACT pipeline starts) with projection fills;
            # kt 0..3 may have been pre-emitted during the previous round
            for kt in range(nkt):
                if kt not in prs0:
                    emit_scores(0, qc, kt, prs0)
                if fills:
                    fills.pop(0)()
            # pair1 scores keep ACT busy; pair0 AV blocks + fills cover PE.
            # Fill BEFORE the AV pops: in round 0 the v_tiles are fills of
            # this very loop and the AV blocks consume them in order.
            ot0 = pot.tile([128, 1024], f32, tag="ot", name="ot_ps")
            avq = [(hh, qb) for qb in range(4) for hh in range(2)]
            for kt in range(nkt):
                emit_scores(1, qc, kt, prs1)
                if fills:
                    fills.pop(0)()
                for _ in range(2 if nkt <= 4 else 1):
                    if avq:
                        hh, qb = avq.pop(0)
                        av_block(0, qc, ot0, prs0, hh, qb)
            while avq:
                hh, qb = avq.pop(0)
                av_block(0, qc, ot0, prs0, hh, qb)
            norm_transpose(0, qc, ot0)
            # pair1 AV blocks with remaining fills; pre-emit the NEXT
            # round's first pair0 score tiles so ACT's exp stream doesn't
            # starve across the round boundary
            ot1 = pot.tile([128, 1024], f32, tag="ot", name="ot_ps")
            pre_next = {}
            for qb in range(4):
                for hh in range(2):
                    av_block(1, qc, ot1, prs1, hh, qb)
                    if fills:
                        fills.pop(0)()
                    if qc < NQC - 1 and len(pre_next) < 4:
                        emit_scores(0, qc + 1, len(pre_next), pre_next)
            norm_transpose(1, qc, ot1, pe_transpose=(qc == NQC - 1))
            for f in fills:
                f()
            prs0 = pre_next

        for ct in range(8):
            zp_step(NQC - 1, ct, epilogue=True)

    nc.compile()
    return nc


def _get_program():
    if "nc" not in _CACHE:
        _CACHE["nc"] = _build_program()
    return _CACHE["nc"]


def _dr_split8(a, nkp):
    """[D0, n] f32 -> [D0//2, 2, 2, n] e4m3: row P*128+p, slot i, comp c
    holds hi/lo of a[256P + 128i + p]."""
    e4m3 = ml_dtypes.float8_e4m3
    hi = a.astype(e4m3)
    lo = (a - hi.astype(np.float32)).astype(e4m3)
    n = a.shape[1]
    out = np.stack([hi, lo], axis=1).reshape(nkp, 2, 128, 2, n)
    return np.ascontiguousarray(out.transpose(0, 2, 1, 3, 4)).reshape(nkp * 128, 2, 2, n)


def _make_in_maps(x, w_qkv, w_out):
    bf = ml_dtypes.bfloat16
    # probs layout [key, query]: keep q >= k (upper triangle incl diagonal)
    mi = np.concatenate([np.triu(np.ones((128, 128), dtype=np.float32), 0),
                         np.eye(128, dtype=np.float32)], axis=1).astype(bf)
    x8b = [_dr_split8(np.ascontiguousarray(x[b].T), NKP) for b in range(B)]
    in_maps = []
    for c in range(NCORES):
        b, g = c // 4, c % 4
        cs = slice(GD * g, GD * (g + 1))
        wqk = np.concatenate(
            [w_qkv[:, cs], w_qkv[:, D + GD * g:D + GD * (g + 1)]], axis=1
        ) * np.float32(WS)
        wv = w_qkv[:, 2 * D + GD * g:2 * D + GD * (g + 1)] * np.float32(WS)
        wos = w_out[cs, :] * np.float32(WS)
        in_maps.append(
            {"x8": x8b[b], "wqk8": _dr_split8(wqk, NKP),
             "wv8": _dr_split8(wv, NKP), "wo8": _dr_split8(wos, 1),
             "wo": np.ascontiguousarray(w_out[cs, :]).astype(bf) * bf(WS),
             "mi": mi})
    return in_maps


def kernel(x, w_qkv, b_qkv, w_out, b_out):
    from concourse.bass_utils import run_bass_kernel_spmd

    x = np.asarray(x, dtype=np.float32)
    w_qkv = np.asarray(w_qkv, dtype=np.float32)
    w_out = np.asarray(w_out, dtype=np.float32)

    nc = _get_program()
    in_maps = _make_in_maps(x, w_qkv, w_out)
    res = run_bass_kernel_spmd(nc, in_maps, list(range(NCORES))).results

    # unshard: sum the 4 TP partial z^T contributions per batch, divide out
    # the weight pre-scales (v-path 64 * w_out-path 64), transpose
    out = np.empty((B, S, D), dtype=np.float32)
    for b in range(B):
        acc = np.zeros((D, S), dtype=np.float32)
        for g in range(4):
            acc += res[4 * b + g]["ztp"].astype(np.float32)
        out[b] = acc.T * np.float32(1.0 / (WS * WS))
    return out


# revision 58
# speedup vs baseline: 1.0004x; 1.0004x over previous
"""GPT2 self-attention on 8 trn2 NeuronCores (tensor-parallel).

Sharding: core c handles batch b = c//4 and head-group g = c%4 (4 of 16
heads = 256 of 1024 dims).

Per core:
  1. Q/K projection in fp8e4 DoubleRow (4x PE throughput): host splits
     x and 64*w_qkv into (hi, lo) e4m3 pairs; q/k = sum of the three
     component products hi*hi + hi*lo + lo*hi, each a DoubleRow matmul
     contracting two 128-d tiles at once. qkt [512 qk-dims, 2048 tokens].
  2. V projection likewise: [2048 tokens, 256 v-dims] (x^T tile as
     DoubleRow lhsT), stored per key-tile as [128, head, 65] with a ones
     column (col 64). v carries the 64x weight scale.
  3. Causal attention per head-pair in bf16, keys on PSUM partitions:
       S^T = K-tile.T @ Q-chunk (both heads into one 2-bank PSUM tile)
       -> merged exp(S/(8*64^2)) on ACT -> diag mask mult on GPSIMD
       -> probs bf16
       AV flipped: out[q-block 128, 65] += probs-block.T @ [V | 1]
       (col 64 = softmax denominator, landing per-query-partition)
     Normalize via DVE reciprocal + per-block tensor_scalar multiply.
  4. Transpose O_norm per 128-query block via DMA-transpose -> O^T [dims, q],
     then DVE-split O^T into (hi, lo) e4m3 for rounds 0-2.
  5. Partial out-projection z^T_partial [1024, 2048]: fp8 DoubleRow
     (both head-pairs in the two slots) for rounds 0-2, bf16 for the
     tail-critical round-3 epilogue. PSUM -> bf16 -> DRAM per [128, 512]
     tile (the output).

Host reorders/slices/casts inputs, and unshards by summing the four
tensor-parallel z^T partials per batch (f32), dividing out the 64^2
weight pre-scale, and transposing into [B, S, D]. b_qkv/b_out are zeros
by the problem spec and are folded out. Attention matmuls run bf16 with
fp32 PSUM accumulation; projections run fp8 DoubleRow with fp32 PSUM
accumulation.
"""

import numpy as np
import ml_dtypes
from contextlib import ExitStack

B, S, D, H = 2, 2048, 1024, 16
HD = 64            # head dim
NCORES = 8
HPC = 4            # heads per core
GD = HPC * HD      # 256 dims per core group
QW = 512           # query-chunk width
WS = 64.0          # weight pre-scale folded into exp-scale / host unshard
NKP = D // 256     # 4 DoubleRow contraction k-pairs over d_model

_CACHE = {}


def _build_program():
    import concourse.tile as tile
    from concourse import bacc, mybir

    bf16 = mybir.dt.bfloat16
    f8 = mybir.dt.float8e4
    f32 = mybir.dt.float32
    DR = mybir.MatmulPerfMode.DoubleRow

    nc = bacc.Bacc("TRN2", target_bir_lowering=False, debug=False,
                   num_devices=NCORES)

    # x8/wqk8/wv8: DoubleRow layouts [kpair*128, slot, comp(hi/lo), n]
    x8 = nc.dram_tensor("x8", [NKP * 128, 2, 2, S], f8, kind="ExternalInput").ap()
    wqk8 = nc.dram_tensor("wqk8", [NKP * 128, 2, 2, 2 * GD], f8, kind="ExternalInput").ap()
    wv8 = nc.dram_tensor("wv8", [NKP * 128, 2, 2, GD], f8, kind="ExternalInput").ap()
    # wo8: [128, slot(GD half), comp, D]; wo: bf16 for the epilogue round
    wo8 = nc.dram_tensor("wo8", [128, 2, 2, D], f8, kind="ExternalInput").ap()
    wo = nc.dram_tensor("wo", [GD, D], bf16, kind="ExternalInput").ap()
    mi = nc.dram_tensor("mi", [128, 256], bf16, kind="ExternalInput").ap()
    ztp = nc.dram_tensor("ztp", [D, S], bf16, kind="ExternalOutput").ap()

    NKT = S // 128          # 16 key tiles
    NQC = S // QW           # 4 query chunks
    ESCALE = 0.125 / (WS * WS)  # exp scale: 1/sqrt(HD) / weight-scale^2

    with tile.TileContext(nc) as tc, ExitStack() as ctx:
        persist = ctx.enter_context(tc.tile_pool(name="persist", bufs=1))
        # PSUM budget (8 banks): pscore 2x2 + pot 1x2 + pmisc 2x1 = 8
        pscore = ctx.enter_context(tc.tile_pool(name="pscore", bufs=2, space="PSUM"))
        pot = ctx.enter_context(tc.tile_pool(name="pot", bufs=1, space="PSUM"))
        pmisc = ctx.enter_context(tc.tile_pool(name="pmisc", bufs=2, space="PSUM"))
        prpool = ctx.enter_context(tc.tile_pool(name="prpool", bufs=36))
        onpool = ctx.enter_context(tc.tile_pool(name="onpool", bufs=3))
        ottpool = ctx.enter_context(tc.tile_pool(name="ottpool", bufs=3))
        recpool = ctx.enter_context(tc.tile_pool(name="recpool", bufs=3))
        zsbpool = ctx.enter_context(tc.tile_pool(name="zsbpool", bufs=10))
        warmpool = ctx.enter_context(tc.tile_pool(name="warmpool", bufs=1))

        x8_sb = [persist.tile([128, 2, 2, S], f8, tag=f"x8{p}", name=f"x8{p}") for p in range(NKP)]
        wqk8_sb = [persist.tile([128, 2, 2, 2 * GD], f8, tag=f"wqk8{p}", name=f"wqk8{p}") for p in range(NKP)]
        wv8_sb = [persist.tile([128, 2, 2, GD], f8, tag=f"wv8{p}", name=f"wv8{p}") for p in range(NKP)]
        wo8_sb = persist.tile([128, 2, 2, D], f8, tag="wo8", name="wo8_sb")
        wo_sb = [persist.tile([128, D], bf16, tag=f"wo{j}", name=f"wo{j}") for j in range(2)]
        mi_sb = persist.tile([128, 256], bf16, tag="mi", name="mi_sb")
        qkt_sb = [persist.tile([128, S], bf16, tag=f"qkt{m}", name=f"qkt{m}") for m in range(4)]
        v_sb = [persist.tile([128, HPC, HD + 1], bf16, tag=f"v{t}", name=f"v{t}") for t in range(NKT)]
        # O^T fp8 hi/lo per round, both pairs in the DoubleRow slot dim
        oth_sb = [persist.tile([128, 2, QW], f8, tag=f"oth{q}", name=f"oth{q}") for q in range(NQC - 1)]
        otl_sb = [persist.tile([128, 2, QW], f8, tag=f"otl{q}", name=f"otl{q}") for q in range(NQC - 1)]
        mtril_sb = mi_sb[:, 0:128]
        ident_sb = mi_sb[:, 128:256]

        # ---- PE p-state warmup ----
        # The cost model ramps PE to full clock only after ~3us of
        # continuous execution. Run throwaway matmuls on a memset tile
        # while the first DMAs land so real work starts near full speed.
        warm = warmpool.tile([128, 512], bf16, tag="warm", name="warm_sb")
        nc.vector.memset(warm[:], 0.0)
        wps = pmisc.tile([128, 512], f32, tag="misc", name="warm_ps")
        for _ in range(5):
            nc.tensor.matmul(wps[:], warm[:, 0:128], warm[:], start=True,
                             stop=True)
        # keep the BIR verifier happy: PSUM must have a reader
        nc.vector.tensor_copy(warm[:, 0:1], wps[:, 0:1])

        # ---- input loads ----
        # First 512 tokens of x + wqk feed qkt_chunk(.,0) immediately.
        # Fan the 8 critical dispatches over all three DMA-capable queues
        # (each dispatch serializes ~790ns on its queue) so the last
        # (x8[p], wqk8[p]) pair lands ~4.5us instead of ~5.5us.
        def xload(p, sl):
            return dict(out=x8_sb[p][:, :, :, sl], in_=x8[p * 128:(p + 1) * 128, :, :, sl])

        def wqkload(p):
            return dict(out=wqk8_sb[p][:], in_=wqk8[p * 128:(p + 1) * 128])

        c0 = slice(0, QW)
        nc.sync.dma_start(**xload(0, c0))
        nc.gpsimd.dma_start(**wqkload(0))
        nc.scalar.dma_start(**xload(1, c0))
        nc.sync.dma_start(**wqkload(1))
        nc.gpsimd.dma_start(**xload(2, c0))
        nc.scalar.dma_start(**wqkload(2))
        nc.sync.dma_start(**xload(3, c0))
        nc.gpsimd.dma_start(**wqkload(3))
        nc.scalar.dma_start(out=mi_sb[:], in_=mi[:])
        for p in range(NKP):
            nc.sync.dma_start(**xload(p, slice(QW, S)))
            nc.gpsimd.dma_start(out=wv8_sb[p][:], in_=wv8[p * 128:(p + 1) * 128])
        nc.gpsimd.dma_start(out=wo8_sb[:], in_=wo8[:])
        for j in range(2):
            nc.gpsimd.dma_start(out=wo_sb[j][:], in_=wo[j * 128:(j + 1) * 128, :])

        # ---- projection helpers (PE fill work, fp8 DoubleRow) ----
        # terms: (w_hi, x_hi), (w_hi, x_lo), (w_lo, x_hi)
        TERMS = [(0, 0), (0, 1), (1, 0)]

        def qkt_chunk(m, n, lo=0, hi=QW, act_copy=False):
            ps = pmisc.tile([128, QW], f32, tag="misc", name="qkt_ps")[:, 0:hi - lo]
            k = 0
            for wc, xc in TERMS:
                for p in range(NKP):
                    nc.tensor.matmul(
                        ps,
                        wqk8_sb[p][:, :, wc, m * 128:(m + 1) * 128],
                        x8_sb[p][:, :, xc, n * QW + lo:n * QW + hi],
                        start=(k == 0), stop=(k == 3 * NKP - 1),
                        perf_mode=DR,
                    )
                    k += 1
            if act_copy:
                # preamble only: ACT is idle pre-exp, so the two first
                # chunk copies run in parallel on DVE + ACT
                nc.scalar.activation(qkt_sb[m][:, n * QW + lo:n * QW + hi],
                                     ps, mybir.ActivationFunctionType.Copy)
            else:
                nc.vector.tensor_copy(qkt_sb[m][:, n * QW + lo:n * QW + hi], ps)

        def v_tile(t):
            ps = pmisc.tile([128, GD], f32, tag="misc", name="v_ps")
            k = 0
            for wc, xc in TERMS:
                for p in range(NKP):
                    nc.tensor.matmul(
                        ps[:, 0:GD],
                        x8_sb[p][:, :, xc, t * 128:(t + 1) * 128],
                        wv8_sb[p][:, :, wc, :],
                        start=(k == 0), stop=(k == 3 * NKP - 1),
                        perf_mode=DR,
                    )
                    k += 1
            nc.vector.tensor_copy(
                v_sb[t][:, :, 0:HD],
                ps[:, 0:GD].rearrange("p (h d) -> p h d", h=HPC),
            )
            nc.vector.memset(v_sb[t][:, :, HD:HD + 1], 1.0)

        ott_of = {}
        zsplit = {}

        def zp_step(qc, ct, epilogue=False):
            """One out-proj column tile: z^T[ct*128:+128, qc*512:+512]."""
            if epilogue:
                if ct in zsplit:
                    # pair0 contribution was accumulated mid-round; only
                    # pair1 remains on the tail-critical path
                    ps = zsplit[ct]
                    nc.tensor.matmul(
                        ps,
                        wo_sb[1][:, ct * 128:(ct + 1) * 128],
                        ott_of[(1, qc)][:],
                        start=False, stop=True,
                    )
                else:
                    pool, tag = ((pmisc, "misc") if ct % 2 == 0
                                 else (pscore, "sc"))
                    ps = pool.tile([128, QW], f32, tag=tag, name="zp_ps")[:]
                    for pair in (0, 1):
                        nc.tensor.matmul(
                            ps,
                            wo_sb[pair][:, ct * 128:(ct + 1) * 128],
                            ott_of[(pair, qc)][:],
                            start=(pair == 0), stop=(pair == 1),
                        )
            else:
                ps = pmisc.tile([128, QW], f32, tag="misc", name="zp_ps")[:]
                # fp8 DoubleRow: both pairs in the slots, 3 hi/lo terms
                zterms = [(0, oth_sb[qc]), (0, otl_sb[qc]), (1, oth_sb[qc])]
                for k, (wc, ot8) in enumerate(zterms):
                    nc.tensor.matmul(
                        ps,
                        wo8_sb[:, :, wc, ct * 128:(ct + 1) * 128],
                        ot8[:],
                        start=(k == 0), stop=(k == 2),
                        perf_mode=DR,
                    )
            zsb = zsbpool.tile([128, QW], bf16, tag="zsb", name="zsb")
            if epilogue and ct % 2 == 0:
                nc.scalar.activation(zsb[:], ps,
                                     mybir.ActivationFunctionType.Copy)
            else:
                nc.vector.tensor_copy(zsb[:], ps)
            if epilogue:
                eng = (nc.sync, nc.gpsimd, nc.scalar, nc.sync)[ct % 4]
            else:
                eng = nc.gpsimd if ct % 2 == 0 else nc.sync
            eng.dma_start(
                out=ztp[ct * 128:(ct + 1) * 128, qc * QW:(qc + 1) * QW],
                in_=zsb[:])

        # ---- attention ----
        def emit_scores(pair, qc, kt, prs):
            qstart = qc * QW
            j = kt - 4 * qc
            qoff = max(0, 128 * j)
            sp = pscore.tile([128, 1024], f32, tag="sc", name="sc_ps")
            pr = prpool.tile([128, 1024], bf16, tag="pr", name="pr_sb")
            for hh in range(2):
                base = 64 * hh
                nc.tensor.matmul(
                    sp[:, 512 * hh + qoff:512 * hh + 512],
                    qkt_sb[2 + pair][base:base + 64, kt * 128:(kt + 1) * 128],
                    qkt_sb[pair][base:base + 64, qstart + qoff:qstart + QW],
                    start=True, stop=True,
                )
            sp3 = sp[:].rearrange("p (h q) -> p h q", h=2)
            pr3 = pr[:].rearrange("p (h q) -> p h q", h=2)
            nc.scalar.activation(
                pr3[:, :, qoff:QW], sp3[:, :, qoff:QW],
                mybir.ActivationFunctionType.Exp,
                scale=ESCALE,
            )
            if j >= 0:
                # causal mask: zero future-key probs in the diagonal tile
                # (on GPSIMD: keeps DVE free for copies/norms)
                nc.gpsimd.tensor_tensor(
                    out=pr3[:, :, qoff:qoff + 128],
                    in0=pr3[:, :, qoff:qoff + 128],
                    in1=mtril_sb.rearrange("p (o c) -> p o c", o=1).to_broadcast([128, 2, 128]),
                    op=mybir.AluOpType.mult,
                )
            prs[kt] = pr

        def av_block(pair, qc, ot, prs, hh, qb):
            """One (head, query-block) AV accumulation group: consecutive
            matmuls over its key tiles (one open PSUM group per bank)."""
            blk = hh * 4 + qb
            last = 4 * qc + qb
            for kt in range(last + 1):
                pr3 = prs[kt][:].rearrange("p (h q) -> p h q", h=2)
                nc.tensor.matmul(
                    ot[:, 128 * blk:128 * blk + HD + 1],
                    pr3[:, hh, qb * 128:(qb + 1) * 128],
                    v_sb[kt][:, 2 * pair + hh, :],
                    start=(kt == 0), stop=(kt == last),
                )

        def norm_transpose(pair, qc, ot, pe_transpose=False):
            ot3 = ot[:].rearrange("p (b q) -> p b q", b=8)
            rec = recpool.tile([128, 8], f32, tag="rec", name="rec_sb")
            nc.vector.reciprocal(rec[:], ot3[:, :, HD:HD + 1])
            onorm = onpool.tile([128, QW], bf16, tag="on", name="on_sb")
            nc.vector.tensor_tensor(
                out=onorm[:].rearrange("p (qb hh d) -> p hh qb d", qb=4, hh=2),
                in0=ot[:].rearrange("p (hh qb c) -> p hh qb c", hh=2, qb=4)[:, :, :, 0:HD],
                in1=rec[:].rearrange("p (hh qb) -> p hh qb", hh=2).to_broadcast([128, 2, 4, HD]),
                op=mybir.AluOpType.mult,
            )
            ott = ottpool.tile([128, QW], bf16, tag=f"ott{pair}", name="ott_sb")
            if pe_transpose:
                # tail-critical: PE is idle here and skips the DMA-queue
                # latency
                tp = pmisc.tile([128, QW], bf16, tag="misc", name="tp_ps")
                for qb in range(4):
                    nc.tensor.transpose(
                        tp[:, qb * 128:(qb + 1) * 128],
                        onorm[:, qb * 128:(qb + 1) * 128],
                        ident_sb)
                nc.vector.tensor_copy(ott[:], tp[:])
            else:
                for qb in range(4):
                    nc.sync.dma_start_transpose(
                        ott[:, qb * 128:(qb + 1) * 128],
                        onorm[:, qb * 128:(qb + 1) * 128])
            ott_of[(pair, qc)] = ott
            if qc < NQC - 1:
                # fp8 hi/lo split of O^T for the DoubleRow out-projection
                nc.vector.tensor_copy(oth_sb[qc][:, pair, :], ott[:])
                nc.vector.tensor_tensor(
                    out=otl_sb[qc][:, pair, :],
                    in0=ott[:],
                    in1=oth_sb[qc][:, pair, :],
                    op=mybir.AluOpType.subtract,
                )

        # ---- main schedule ----
        qkt_chunk(0, 0)
        qkt_chunk(2, 0)

        prs0 = {}
        for qc in range(NQC):
            nkt = 4 * qc + 4
            fills = []
            if qc == 0:
                fills.append(lambda: qkt_chunk(1, 0))
                fills.append(lambda: qkt_chunk(3, 0))
                for t in range(4):
                    fills.append(lambda t=t: v_tile(t))
                fills.append(lambda: qkt_chunk(0, 1))
                fills.append(lambda: qkt_chunk(2, 1))
            else:
                # v tiles for THIS round's AV phase: consumed during pair0
                # scores
                for t in range(4 * qc, 4 * qc + 4):
                    fills.append(lambda t=t: v_tile(t))
                if qc < NQC - 1:
                    fills.append(lambda n=qc + 1: qkt_chunk(0, n))
                    fills.append(lambda n=qc + 1: qkt_chunk(2, n))
            if qc == NQC - 1:
                fills.append(lambda n=qc: qkt_chunk(1, n))
                fills.append(lambda n=qc: qkt_chunk(3, n))
            elif qc < NQC - 2:
                fills.append(lambda n=qc + 1: qkt_chunk(1, n))
                fills.append(lambda n=qc + 1: qkt_chunk(3, n))
            # all zp waves deferred to round 3 — the only PE-starved round
            if qc == NQC - 1:
                for q in range(NQC - 1):
                    for ct in range(8):
                        fills.append(lambda c=ct, q=q: zp_step(q, c))

            prs1 = {}
            # pair0 scores (ACT pipeline starts) with projection fills;
            # kt 0..3 may have been pre-emitted during the previous round
            for kt in range(nkt):
                if kt not in prs0:
                    emit_scores(0, qc, kt, prs0)
                if fills:
                    fills.pop(0)()
            # pair1 scores keep ACT busy; pair0 AV blocks + fills cover PE.
            # Fill BEFORE the AV pops: in round 0 the v_tiles are fills of
            # this very loop and the AV blocks consume them in order.
            ot0 = pot.tile([128, 1024], f32, tag="ot", name="ot_ps")
            avq = [(hh, qb) for qb in range(4) for hh in range(2)]
            for kt in range(nkt):
                emit_scores(1, qc, kt, prs1)
                if fills:
                    fills.pop(0)()
                for _ in range(2 if nkt <= 4 else 1):
                    if avq:
                        hh, qb = avq.pop(0)
                        av_block(0, qc, ot0, prs0, hh, qb)
            while avq:
                hh, qb = avq.pop(0)
                av_block(0, qc, ot0, prs0, hh, qb)
            norm_transpose(0, qc, ot0)
            # pair1 AV blocks with remaining fills; pre-emit the NEXT
            # round's first pair0 score tiles so ACT's exp stream doesn't
            # starve across the round boundary
            ot1 = pot.tile([128, 1024], f32, tag="ot", name="ot_ps")
            pre_next = {}
            for qb in range(4):
                for hh in range(2):
                    av_block(1, qc, ot1, prs1, hh, qb)
                    if fills:
                        fills.pop(0)()
                    if qc < NQC - 1 and len(pre_next) < 4:
                        emit_scores(0, qc + 1, len(pre_next), pre_next)
            norm_transpose(1, qc, ot1, pe_transpose=(qc == NQC - 1))
            for f in fills:
                f()
            prs0 = pre_next

        for ct in range(8):
            zp_step(NQC - 1, ct, epilogue=True)

    nc.compile()
    return nc


def _get_program():
    if "nc" not in _CACHE:
        _CACHE["nc"] = _build_program()
    return _CACHE["nc"]


def _dr_split8(a, nkp):
    """[D0, n] f32 -> [D0//2, 2, 2, n] e4m3: row P*128+p, slot i, comp c
    holds hi/lo of a[256P + 128i + p]."""
    e4m3 = ml_dtypes.float8_e4m3
    hi = a.astype(e4m3)
    lo = (a - hi.astype(np.float32)).astype(e4m3)
    n = a.shape[1]
    out = np.stack([hi, lo], axis=1).reshape(nkp, 2, 128, 2, n)
    return np.ascontiguousarray(out.transpose(0, 2, 1, 3, 4)).reshape(nkp * 128, 2, 2, n)


def _make_in_maps(x, w_qkv, w_out):
    bf = ml_dtypes.bfloat16
    # probs layout [key, query]: keep q >= k (upper triangle incl diagonal)
    mi = np.concatenate([np.triu(np.ones((128, 128), dtype=np.float32), 0),
                         np.eye(128, dtype=np.float32)], axis=1).astype(bf)
    x8b = [_dr_split8(np.ascontiguousarray(x[b].T), NKP) for b in range(B)]
    in_maps = []
    for c in range(NCORES):
        b, g = c // 4, c % 4
        cs = slice(GD * g, GD * (g + 1))
        wqk = np.concatenate(
            [w_qkv[:, cs], w_qkv[:, D + GD * g:D + GD * (g + 1)]], axis=1
        ) * np.float32(WS)
        wv = w_qkv[:, 2 * D + GD * g:2 * D + GD * (g + 1)] * np.float32(WS)
        wos = w_out[cs, :] * np.float32(WS)
        in_maps.append(
            {"x8": x8b[b], "wqk8": _dr_split8(wqk, NKP),
             "wv8": _dr_split8(wv, NKP), "wo8": _dr_split8(wos, 1),
             "wo": np.ascontiguousarray(w_out[cs, :]).astype(bf) * bf(WS),
             "mi": mi})
    return in_maps


def kernel(x, w_qkv, b_qkv, w_out, b_out):
    from concourse.bass_utils import run_bass_kernel_spmd

    x = np.asarray(x, dtype=np.float32)
    w_qkv = np.asarray(w_qkv, dtype=np.float32)
    w_out = np.asarray(w_out, dtype=np.float32)

    nc = _get_program()
    in_maps = _make_in_maps(x, w_qkv, w_out)
    res = run_bass_kernel_spmd(nc, in_maps, list(range(NCORES))).results

    # unshard: sum the 4 TP partial z^T contributions per batch, divide out
    # the weight pre-scales (v-path 64 * w_out-path 64), transpose
    out = np.empty((B, S, D), dtype=np.float32)
    for b in range(B):
        acc = np.zeros((D, S), dtype=np.float32)
        for g in range(4):
            acc += res[4 * b + g]["ztp"].astype(np.float32)
        out[b] = acc.T * np.float32(1.0 / (WS * WS))
    return out
